# revision 33
# baseline (speedup 1.0000x reference)
"""Trainium2 Bass kernel for nn_BaseSparseVAE.

Reference computation (B=256, D=512, L=64, H=300):
    h  = relu(x @ enc1_w.T + enc1_b)                 # [B, H]
    h  = relu(h @ enc2_w.T + enc2_b)                 # [B, H]
    z_mean    = h @ zm_w.T + zm_b                    # [B, L]
    z_log_var = h @ zv_w.T + zv_b                    # [B, L]
    z  = z_mean + eps * exp(0.5 * z_log_var)         # [B, L]
    masked = z[:, None, :] * W[None, :, :]           # [B, D, L]
    g1 = relu(einsum('bdl,hl->bdh', masked, gen1_w)) # [B, D, H]
    g2 = relu(einsum('bdh,gh->bdg', g1, gen2_w) + gen2_b)
    x_mean = einsum('bdh,dh->bd', g2, head_w) + head_b
    returns (x_mean, z, z_mean, z_log_var)

Sharding: decode dominates (~28.7 of 28.8 GFLOP) and is embarrassingly
parallel over the D axis -> each of the 8 cores owns 64 columns of D
(W rows + head_w rows sharded, generator weights replicated). The tiny
encoder is replicated on every core, which avoids all collectives.

All matmuls run in fp32r (1 PE cycle/row vs 4 for fp32; measured rel err
~2e-4 vs 3e-3 for bf16). fp32r operands must be produced by a compute op
that rounds to the fp32r format, so DMA-loaded operands pass through one
DVE conversion copy and every activation/bias op writes fp32r directly.

On-chip layout keeps features on partitions and batch on the free dim,
so every matmul bias is a per-partition scalar:
    h1T/h2T [300, 256], zT [64, 256],
    per column-block of 512 (= 2 d-columns x 256 batch):
        maskedT [64, 512] -> g1T [300, 512] -> g2T [300, 512]
        head psum [64, 512] (all 64 local d rows x this block), of which
        the 2 matching diagonal rows are DMA'd to the x_meanT output.
head_b is added on the host during the unshard (epilogue, O(B*D)).
"""

import os
import sys

import numpy as np

for _p in ("/opt/trn_rl_repo",):
    if os.path.isdir(_p) and _p not in sys.path:
        sys.path.insert(0, _p)

import concourse.bacc as bacc
import concourse.mybir as mybir
import concourse.tile as tile
from concourse import bass_utils as _bu
from concourse.bass_utils import run_bass_kernel_spmd

# fp32r matmuls reload the stationary operand on every matmul (no FWL for
# 4-byte weights) which serializes ~110ns per matmul; walrus's ldw-opt
# elides the redundant reloads. It is off by default in this stack, so
# rewrite the flag on the walrus invocation.
if os.environ.get("LDW_OPT", "0") == "1" and not getattr(_bu, "_ldwopt_patched", False):
    _orig_run_command = _bu.run_command

    def _run_command_ldwopt(argv, **kw):
        argv = [
            "--enable-ldw-opt=true" if a == "--enable-ldw-opt=false" else a
            for a in argv
        ]
        return _orig_run_command(argv, **kw)

    _bu.run_command = _run_command_ldwopt
    _bu._ldwopt_patched = True

B, D, L, H = 256, 512, 64, 300
NCORES = 8
DL = D // NCORES  # d-columns per core

F32 = mybir.dt.float32
F32R = mybir.dt.float32r
BF16 = mybir.dt.bfloat16
AF = mybir.ActivationFunctionType
ALU = mybir.AluOpType

# BF16_DECODE=1 runs the decode stage (masked/g1/g2/head tiles + their
# weights) in bf16 instead of fp32r: FWL halves the per-matmul weight-load
# at the cost of decode accuracy (~2e-3 vs ~2e-4).
DEC = BF16 if os.environ.get("BF16_DECODE", "0") == "1" else F32R

MT = [(0, 128), (128, 128), (256, 44)]  # partition tiles of H=300
KT512 = [(k * 128, 128) for k in range(4)]  # partition tiles of D=512

NB = 512  # matmul moving free dim (one PSUM bank of fp32)
DPB = NB // B  # d-columns per block = 2
NBLOCKS = DL // DPB  # 32


def build_program():
    nc = bacc.Bacc(trn_type="TRN2")

    def din(name, shape):
        return nc.dram_tensor(name, shape, F32, kind="ExternalInput").ap()

    def dout(name, shape):
        return nc.dram_tensor(name, shape, F32, kind="ExternalOutput").ap()

    xT = din("xT", [D, B])
    epsT = din("epsT", [L, B])
    enc1_wT = din("enc1_wT", [D, H])
    enc1_b = din("enc1_b", [H, 1])
    enc2_wT = din("enc2_wT", [H, H])
    enc2_b = din("enc2_b", [H, 1])
    zm_wT = din("zm_wT", [H, L])
    zm_b = din("zm_b", [L, 1])
    zv_wT = din("zv_wT", [H, L])
    zv_b = din("zv_b", [L, 1])
    gen1_wT = din("gen1_wT", [L, H])
    gen2_wT = din("gen2_wT", [H, H])
    gen2_b = din("gen2_b", [H, 1])
    w_lT = din("W_lT", [L, DL])
    head_wT = din("head_wT", [H, DL])

    x_meanT = dout("x_meanT", [DL, B])
    zT_out = dout("zT", [L, B])
    z_meanT_out = dout("z_meanT", [L, B])
    z_log_varT_out = dout("z_log_varT", [L, B])

    with tile.TileContext(nc) as tc:
        with (
            tc.tile_pool(name="consts", bufs=1) as consts,
            tc.tile_pool(name="ldtmp", bufs=8) as ldtmp,
            tc.tile_pool(name="enc", bufs=1) as enc,
            tc.tile_pool(name="mask", bufs=6) as maskp,
            tc.tile_pool(name="g1", bufs=3) as g1p,
            tc.tile_pool(name="g2", bufs=3) as g2p,
            tc.tile_pool(name="hstg", bufs=4) as hstp,
            tc.tile_pool(name="psA", bufs=4, space="PSUM") as psA,
            tc.tile_pool(name="psB", bufs=3, space="PSUM") as psB,
            tc.tile_pool(name="psH", bufs=1, space="PSUM") as psH,
        ):

            def load_f32(src, shape, tag):
                # plain fp32 resident tile (bias / DVE-scalar operands)
                t = consts.tile(shape, F32, tag=tag)
                nc.sync.dma_start(out=t, in_=src)
                return t

            def load_r(src, shape, tag, dtype=F32R):
                # resident tile for matmul operands: DMA the raw fp32 then
                # round to fp32r/bf16 with a DVE copy (required by the PE)
                tmp = ldtmp.tile(shape, F32, tag="ldtmp")
                nc.sync.dma_start(out=tmp, in_=src)
                t = consts.tile(shape, dtype, tag=tag)
                nc.vector.tensor_copy(t, tmp)
                return t

            # ---- resident inputs -------------------------------------
            xT_sb = [
                load_r(xT[ko : ko + ki, :], [ki, B], f"xT{j}")
                for j, (ko, ki) in enumerate(KT512)
            ]
            e1w_sb = [
                load_r(enc1_wT[ko : ko + ki, :], [ki, H], f"e1w{j}")
                for j, (ko, ki) in enumerate(KT512)
            ]
            e2w_sb = [
                load_r(enc2_wT[ko : ko + ki, :], [ki, H], f"e2w{j}")
                for j, (ko, ki) in enumerate(MT)
            ]
            zmw_sb = [
                load_r(zm_wT[ko : ko + ki, :], [ki, L], f"zmw{j}")
                for j, (ko, ki) in enumerate(MT)
            ]
            zvw_sb = [
                load_r(zv_wT[ko : ko + ki, :], [ki, L], f"zvw{j}")
                for j, (ko, ki) in enumerate(MT)
            ]
            g1w_sb = load_r(gen1_wT, [L, H], "g1w", DEC)
            g2w_sb = [
                load_r(gen2_wT[ko : ko + ki, :], [ki, H], f"g2w{j}", DEC)
                for j, (ko, ki) in enumerate(MT)
            ]
            hw_sb = [
                load_r(head_wT[ko : ko + ki, :], [ki, DL], f"hw{j}", DEC)
                for j, (ko, ki) in enumerate(MT)
            ]
            e1b_sb = [
                load_f32(enc1_b[mo : mo + mi, :], [mi, 1], f"e1b{i}")
                for i, (mo, mi) in enumerate(MT)
            ]
            e2b_sb = [
                load_f32(enc2_b[mo : mo + mi, :], [mi, 1], f"e2b{i}")
                for i, (mo, mi) in enumerate(MT)
            ]
            g2b_sb = [
                load_f32(gen2_b[mo : mo + mi, :], [mi, 1], f"g2b{i}")
                for i, (mo, mi) in enumerate(MT)
            ]
            zmb_sb = load_f32(zm_b, [L, 1], "zmb")
            zvb_sb = load_f32(zv_b, [L, 1], "zvb")
            epsT_sb = load_f32(epsT, [L, B], "epsT")
            wl_sb = load_f32(w_lT, [L, DL], "wl")

            # ---- encoder (replicated on every core) ------------------
            h1_sb = []
            for i, (mo, mi) in enumerate(MT):
                ps = psA.tile([mi, B], F32, tag="psA")
                for j, (ko, ki) in enumerate(KT512):
                    nc.tensor.matmul(
                        ps, lhsT=e1w_sb[j][:, mo : mo + mi], rhs=xT_sb[j],
                        start=(j == 0), stop=(j == len(KT512) - 1))
                h = enc.tile([mi, B], F32R, tag=f"h1_{i}")
                nc.scalar.activation(h, ps, AF.Relu, bias=e1b_sb[i])
                h1_sb.append(h)

            h2_sb = []
            for i, (mo, mi) in enumerate(MT):
                ps = psA.tile([mi, B], F32, tag="psA")
                for j, (ko, ki) in enumerate(MT):
                    nc.tensor.matmul(
                        ps, lhsT=e2w_sb[j][:, mo : mo + mi], rhs=h1_sb[j],
                        start=(j == 0), stop=(j == len(MT) - 1))
                h = enc.tile([mi, B], F32R, tag=f"h2_{i}")
                nc.scalar.activation(h, ps, AF.Relu, bias=e2b_sb[i])
                h2_sb.append(h)

            ps_zm = psB.tile([L, B], F32, tag="psB")
            for j, (ko, ki) in enumerate(MT):
                nc.tensor.matmul(ps_zm, lhsT=zmw_sb[j], rhs=h2_sb[j],
                                 start=(j == 0), stop=(j == len(MT) - 1))
            zmean_sb = enc.tile([L, B], F32, tag="zmean")
            nc.vector.tensor_scalar_add(zmean_sb, ps_zm, zmb_sb)
            nc.sync.dma_start(out=z_meanT_out, in_=zmean_sb)

            ps_zv = psB.tile([L, B], F32, tag="psB")
            for j, (ko, ki) in enumerate(MT):
                nc.tensor.matmul(ps_zv, lhsT=zvw_sb[j], rhs=h2_sb[j],
                                 start=(j == 0), stop=(j == len(MT) - 1))
            zlv_sb = enc.tile([L, B], F32, tag="zlv")
            nc.vector.tensor_scalar_add(zlv_sb, ps_zv, zvb_sb)
            nc.sync.dma_start(out=z_log_varT_out, in_=zlv_sb)

            e_sb = enc.tile([L, B], F32, tag="e")
            nc.scalar.activation(e_sb, zlv_sb, AF.Exp, scale=0.5)
            prod_sb = enc.tile([L, B], F32, tag="prod")
            nc.vector.tensor_mul(prod_sb, e_sb, epsT_sb)
            z_sb = enc.tile([L, B], F32, tag="z")
            nc.vector.tensor_add(z_sb, prod_sb, zmean_sb)
            nc.sync.dma_start(out=zT_out, in_=z_sb)

            # ---- decode: 32 blocks of 2 d-columns x 256 batch --------
            # Per-block order g1 -> g2 -> head, but each block's maskedT is
            # produced at the END of the previous block so ScalarE services
            # the g1 relus first and the mask hides under g2/head matmuls.
            mk_store = {}

            def do_mask(blk):
                # maskedT = z * W-column, per-partition scale on ScalarE
                mk = maskp.tile([L, NB], DEC, tag="mask")
                for t in range(DPB):
                    d = blk * DPB + t
                    # VectorE 2x-mode tensor_scalar: ~2.5x cheaper than the
                    # ScalarE scaled-copy, and keeps ScalarE free for relus
                    nc.vector.tensor_scalar_mul(
                        mk[:, t * B : (t + 1) * B], z_sb, wl_sb[:, d : d + 1]
                    )
                mk_store[blk] = mk

            def do_g1(blk):
                mk = mk_store.pop(blk)
                g1_sb = []
                for i, (mo, mi) in enumerate(MT):
                    ps = psA.tile([mi, NB], F32, tag="psA")
                    nc.tensor.matmul(ps, lhsT=g1w_sb[:, mo : mo + mi], rhs=mk,
                                     start=True, stop=True)
                    t_ = g1p.tile([mi, NB], DEC, tag=f"g1_{i}")
                    nc.scalar.activation(t_, ps, AF.Relu)
                    g1_sb.append(t_)
                return g1_sb

            g2_store = {}

            def do_g2(blk, g1_sb):
                g2_sb = []
                for i, (mo, mi) in enumerate(MT):
                    ps = psB.tile([mi, NB], F32, tag="psB")
                    for j, (ko, ki) in enumerate(MT):
                        nc.tensor.matmul(
                            ps, lhsT=g2w_sb[j][:, mo : mo + mi], rhs=g1_sb[j],
                            start=(j == 0), stop=(j == len(MT) - 1))
                    t_ = g2p.tile([mi, NB], DEC, tag=f"g2_{i}")
                    # relu(psum + gen2_b) on the vector engine
                    nc.vector.tensor_scalar(
                        t_, ps, g2b_sb[i], 0.0, op0=ALU.add, op1=ALU.max
                    )
                    g2_sb.append(t_)
                g2_store[blk] = g2_sb

            def do_head(blk):
                g2_sb = g2_store.pop(blk)
                psh = psH.tile([DL, NB], F32, tag="psH")
                for j, (ko, ki) in enumerate(MT):
                    nc.tensor.matmul(psh, lhsT=hw_sb[j], rhs=g2_sb[j],
                                     start=(j == 0), stop=(j == len(MT) - 1))
                # PSUM is not DMA-readable: stage through SBUF via VectorE,
                # then DMA out the two diagonal rows of this block
                hs = hstp.tile([DL, NB], F32, tag="hstage")
                nc.scalar.activation(hs, psh, AF.Copy)
                for t in range(DPB):
                    d = blk * DPB + t
                    nc.sync.dma_start(
                        out=x_meanT[d : d + 1, :],
                        in_=hs[d : d + 1, t * B : (t + 1) * B],
                    )

            # PE order per iteration: [g1(blk) | head(blk-1) | g2(blk)].
            # head(blk-1)'s ~0.7us of matmuls sit between g1(blk) and
            # g2(blk), exactly covering the g1-relu -> g2-matmul latency;
            # all of head(blk-1)'s inputs are a block old. The next mask
            # is produced after this block's relus on the scalar engine.
            do_mask(0)
            for blk in range(NBLOCKS):
                g1_sb = do_g1(blk)
                if blk + 1 < NBLOCKS:
                    do_mask(blk + 1)
                if blk > 0:
                    do_head(blk - 1)
                do_g2(blk, g1_sb)
            do_head(NBLOCKS - 1)

    nc.compile()
    return nc


_NC_CACHE = None


def _get_nc():
    global _NC_CACHE
    if _NC_CACHE is None:
        _NC_CACHE = build_program()
    return _NC_CACHE


def make_in_maps(inputs):
    f = lambda a: np.ascontiguousarray(np.asarray(a), dtype=np.float32)
    shared = {
        "xT": f(inputs["x"].T),
        "epsT": f(inputs["eps"].T),
        "enc1_wT": f(inputs["enc1_w"].T),
        "enc1_b": f(inputs["enc1_b"].reshape(H, 1)),
        "enc2_wT": f(inputs["enc2_w"].T),
        "enc2_b": f(inputs["enc2_b"].reshape(H, 1)),
        "zm_wT": f(inputs["zm_w"].T),
        "zm_b": f(inputs["zm_b"].reshape(L, 1)),
        "zv_wT": f(inputs["zv_w"].T),
        "zv_b": f(inputs["zv_b"].reshape(L, 1)),
        "gen1_wT": f(inputs["gen1_w"].T),
        "gen2_wT": f(inputs["gen2_w"].T),
        "gen2_b": f(inputs["gen2_b"].reshape(H, 1)),
    }
    W = f(inputs["W"])
    head_w = f(inputs["head_w"])
    in_maps = []
    for c in range(NCORES):
        m = dict(shared)
        m["W_lT"] = f(W[c * DL : (c + 1) * DL, :].T)
        m["head_wT"] = f(head_w[c * DL : (c + 1) * DL, :].T)
        in_maps.append(m)
    return in_maps


def run_spmd(inputs, **kwargs):
    nc = _get_nc()
    return run_bass_kernel_spmd(nc, make_in_maps(inputs), list(range(NCORES)), **kwargs)


def assemble_outputs(results, inputs):
    x_mean = np.concatenate(
        [results[c]["x_meanT"].T for c in range(NCORES)], axis=1
    ) + np.asarray(inputs["head_b"], np.float32)[None, :]
    z_mean = results[0]["z_meanT"].T.copy()
    z_log_var = results[0]["z_log_varT"].T.copy()
    # recompute the reparameterization on host: exact fp32 exp (the on-chip
    # ACT Exp is an approximation; decode used the on-chip z)
    z = z_mean + np.asarray(inputs["eps"], np.float32) * np.exp(
        0.5 * z_log_var
    ).astype(np.float32)
    return x_mean, z, z_mean, z_log_var


def kernel(**inputs):
    res = run_spmd(inputs)
    return assemble_outputs(res.results, inputs)


if __name__ == "__main__":
    rng = np.random.default_rng(0)
    fake = {
        "x": rng.standard_normal((B, D), dtype=np.float32),
        "eps": rng.standard_normal((B, L), dtype=np.float32),
        "W": rng.standard_normal((D, L), dtype=np.float32) * 0.05,
        "enc1_w": rng.standard_normal((H, D), dtype=np.float32) * 0.05,
        "enc1_b": rng.standard_normal((H,), dtype=np.float32) * 0.05,
        "enc2_w": rng.standard_normal((H, H), dtype=np.float32) * 0.05,
        "enc2_b": rng.standard_normal((H,), dtype=np.float32) * 0.05,
        "zm_w": rng.standard_normal((L, H), dtype=np.float32) * 0.05,
        "zm_b": rng.standard_normal((L,), dtype=np.float32) * 0.05,
        "zv_w": rng.standard_normal((L, H), dtype=np.float32) * 0.05,
        "zv_b": rng.standard_normal((L,), dtype=np.float32) * 0.05,
        "gen1_w": rng.standard_normal((H, L), dtype=np.float32) * 0.05,
        "gen2_w": rng.standard_normal((H, H), dtype=np.float32) * 0.05,
        "gen2_b": rng.standard_normal((H,), dtype=np.float32) * 0.05,
        "head_w": rng.standard_normal((D, H), dtype=np.float32) * 0.05,
        "head_b": rng.standard_normal((D,), dtype=np.float32) * 0.05,
    }
    outs = kernel(**fake)
    for name, o in zip(("x_mean", "z", "z_mean", "z_log_var"), outs):
        print(name, o.shape, o.dtype, float(np.abs(o).max()))


# revision 34
# speedup vs baseline: 1.0144x; 1.0144x over previous
"""Trainium2 Bass kernel for nn_BaseSparseVAE.

Reference computation (B=256, D=512, L=64, H=300):
    h  = relu(x @ enc1_w.T + enc1_b)                 # [B, H]
    h  = relu(h @ enc2_w.T + enc2_b)                 # [B, H]
    z_mean    = h @ zm_w.T + zm_b                    # [B, L]
    z_log_var = h @ zv_w.T + zv_b                    # [B, L]
    z  = z_mean + eps * exp(0.5 * z_log_var)         # [B, L]
    masked = z[:, None, :] * W[None, :, :]           # [B, D, L]
    g1 = relu(einsum('bdl,hl->bdh', masked, gen1_w)) # [B, D, H]
    g2 = relu(einsum('bdh,gh->bdg', g1, gen2_w) + gen2_b)
    x_mean = einsum('bdh,dh->bd', g2, head_w) + head_b
    returns (x_mean, z, z_mean, z_log_var)

Sharding: decode dominates (~28.7 of 28.8 GFLOP) and is embarrassingly
parallel over the D axis -> each of the 8 cores owns 64 columns of D
(W rows + head_w rows sharded, generator weights replicated). The tiny
encoder is replicated on every core, which avoids all collectives.

All matmuls run in fp32r (1 PE cycle/row vs 4 for fp32; measured rel err
~2e-4 vs 3e-3 for bf16). fp32r operands must be produced by a compute op
that rounds to the fp32r format, so DMA-loaded operands pass through one
DVE conversion copy and every activation/bias op writes fp32r directly.

On-chip layout keeps features on partitions and batch on the free dim,
so every matmul bias is a per-partition scalar:
    h1T/h2T [300, 256], zT [64, 256],
    per column-block of 512 (= 2 d-columns x 256 batch):
        maskedT [64, 512] -> g1T [300, 512] -> g2T [300, 512]
        head psum [64, 512] (all 64 local d rows x this block), of which
        the 2 matching diagonal rows are DMA'd to the x_meanT output.
head_b is added on the host during the unshard (epilogue, O(B*D)).
"""

import os
import sys

import numpy as np

for _p in ("/opt/trn_rl_repo",):
    if os.path.isdir(_p) and _p not in sys.path:
        sys.path.insert(0, _p)

import concourse.bacc as bacc
import concourse.mybir as mybir
import concourse.tile as tile
from concourse import bass_utils as _bu
from concourse.bass_utils import run_bass_kernel_spmd

# fp32r matmuls reload the stationary operand on every matmul (no FWL for
# 4-byte weights) which serializes ~110ns per matmul; walrus's ldw-opt
# elides the redundant reloads. It is off by default in this stack, so
# rewrite the flag on the walrus invocation.
if os.environ.get("LDW_OPT", "0") == "1" and not getattr(_bu, "_ldwopt_patched", False):
    _orig_run_command = _bu.run_command

    def _run_command_ldwopt(argv, **kw):
        argv = [
            "--enable-ldw-opt=true" if a == "--enable-ldw-opt=false" else a
            for a in argv
        ]
        return _orig_run_command(argv, **kw)

    _bu.run_command = _run_command_ldwopt
    _bu._ldwopt_patched = True

B, D, L, H = 256, 512, 64, 300
NCORES = 8
DL = D // NCORES  # d-columns per core

F32 = mybir.dt.float32
F32R = mybir.dt.float32r
BF16 = mybir.dt.bfloat16
AF = mybir.ActivationFunctionType
ALU = mybir.AluOpType

# BF16_DECODE=1 runs the decode stage (masked/g1/g2/head tiles + their
# weights) in bf16 instead of fp32r: FWL halves the per-matmul weight-load
# at the cost of decode accuracy (~2e-3 vs ~2e-4).
DEC = BF16 if os.environ.get("BF16_DECODE", "0") == "1" else F32R

MT = [(0, 128), (128, 128), (256, 44)]  # partition tiles of H=300
KT512 = [(k * 128, 128) for k in range(4)]  # partition tiles of D=512

NB = 512  # matmul moving free dim (one PSUM bank of fp32)
DPB = NB // B  # d-columns per block = 2
NBLOCKS = DL // DPB  # 32


def build_program():
    nc = bacc.Bacc(trn_type="TRN2")

    def din(name, shape):
        return nc.dram_tensor(name, shape, F32, kind="ExternalInput").ap()

    def dout(name, shape):
        return nc.dram_tensor(name, shape, F32, kind="ExternalOutput").ap()

    xT = din("xT", [D, B])
    epsT = din("epsT", [L, B])
    enc1_wT = din("enc1_wT", [D, H])
    enc1_b = din("enc1_b", [H, 1])
    enc2_wT = din("enc2_wT", [H, H])
    enc2_b = din("enc2_b", [H, 1])
    zm_wT = din("zm_wT", [H, L])
    zm_b = din("zm_b", [L, 1])
    zv_wT = din("zv_wT", [H, L])
    zv_b = din("zv_b", [L, 1])
    gen1_wT = din("gen1_wT", [L, H])
    gen2_wT = din("gen2_wT", [H, H])
    gen2_b = din("gen2_b", [H, 1])
    w_lT = din("W_lT", [L, DL])
    head_wT = din("head_wT", [H, DL])

    x_meanT = dout("x_meanT", [DL, B])
    zT_out = dout("zT", [L, B])
    z_meanT_out = dout("z_meanT", [L, B])
    z_log_varT_out = dout("z_log_varT", [L, B])

    with tile.TileContext(nc) as tc:
        with (
            tc.tile_pool(name="consts", bufs=1) as consts,
            tc.tile_pool(name="ldtmp", bufs=8) as ldtmp,
            tc.tile_pool(name="enc", bufs=1) as enc,
            tc.tile_pool(name="mask", bufs=6) as maskp,
            tc.tile_pool(name="g1", bufs=3) as g1p,
            tc.tile_pool(name="g2", bufs=3) as g2p,
            tc.tile_pool(name="hstg", bufs=4) as hstp,
            tc.tile_pool(name="psA", bufs=3, space="PSUM") as psA,
            tc.tile_pool(name="psB", bufs=3, space="PSUM") as psB,
            tc.tile_pool(name="psH", bufs=2, space="PSUM") as psH,
        ):

            def load_f32(src, shape, tag):
                # plain fp32 resident tile (bias / DVE-scalar operands)
                t = consts.tile(shape, F32, tag=tag)
                nc.sync.dma_start(out=t, in_=src)
                return t

            def load_r(src, shape, tag, dtype=F32R):
                # resident tile for matmul operands: DMA the raw fp32 then
                # round to fp32r/bf16 with a DVE copy (required by the PE)
                tmp = ldtmp.tile(shape, F32, tag="ldtmp")
                nc.sync.dma_start(out=tmp, in_=src)
                t = consts.tile(shape, dtype, tag=tag)
                nc.vector.tensor_copy(t, tmp)
                return t

            # ---- resident inputs -------------------------------------
            xT_sb = [
                load_r(xT[ko : ko + ki, :], [ki, B], f"xT{j}")
                for j, (ko, ki) in enumerate(KT512)
            ]
            e1w_sb = [
                load_r(enc1_wT[ko : ko + ki, :], [ki, H], f"e1w{j}")
                for j, (ko, ki) in enumerate(KT512)
            ]
            e2w_sb = [
                load_r(enc2_wT[ko : ko + ki, :], [ki, H], f"e2w{j}")
                for j, (ko, ki) in enumerate(MT)
            ]
            zmw_sb = [
                load_r(zm_wT[ko : ko + ki, :], [ki, L], f"zmw{j}")
                for j, (ko, ki) in enumerate(MT)
            ]
            zvw_sb = [
                load_r(zv_wT[ko : ko + ki, :], [ki, L], f"zvw{j}")
                for j, (ko, ki) in enumerate(MT)
            ]
            g1w_sb = load_r(gen1_wT, [L, H], "g1w", DEC)
            g2w_sb = [
                load_r(gen2_wT[ko : ko + ki, :], [ki, H], f"g2w{j}", DEC)
                for j, (ko, ki) in enumerate(MT)
            ]
            hw_sb = [
                load_r(head_wT[ko : ko + ki, :], [ki, DL], f"hw{j}", DEC)
                for j, (ko, ki) in enumerate(MT)
            ]
            e1b_sb = [
                load_f32(enc1_b[mo : mo + mi, :], [mi, 1], f"e1b{i}")
                for i, (mo, mi) in enumerate(MT)
            ]
            e2b_sb = [
                load_f32(enc2_b[mo : mo + mi, :], [mi, 1], f"e2b{i}")
                for i, (mo, mi) in enumerate(MT)
            ]
            g2b_sb = [
                load_f32(gen2_b[mo : mo + mi, :], [mi, 1], f"g2b{i}")
                for i, (mo, mi) in enumerate(MT)
            ]
            zmb_sb = load_f32(zm_b, [L, 1], "zmb")
            zvb_sb = load_f32(zv_b, [L, 1], "zvb")
            epsT_sb = load_f32(epsT, [L, B], "epsT")
            wl_sb = load_f32(w_lT, [L, DL], "wl")

            # ---- encoder (replicated on every core) ------------------
            h1_sb = []
            for i, (mo, mi) in enumerate(MT):
                ps = psA.tile([mi, B], F32, tag="psA")
                for j, (ko, ki) in enumerate(KT512):
                    nc.tensor.matmul(
                        ps, lhsT=e1w_sb[j][:, mo : mo + mi], rhs=xT_sb[j],
                        start=(j == 0), stop=(j == len(KT512) - 1))
                h = enc.tile([mi, B], F32R, tag=f"h1_{i}")
                nc.scalar.activation(h, ps, AF.Relu, bias=e1b_sb[i])
                h1_sb.append(h)

            h2_sb = []
            for i, (mo, mi) in enumerate(MT):
                ps = psA.tile([mi, B], F32, tag="psA")
                for j, (ko, ki) in enumerate(MT):
                    nc.tensor.matmul(
                        ps, lhsT=e2w_sb[j][:, mo : mo + mi], rhs=h1_sb[j],
                        start=(j == 0), stop=(j == len(MT) - 1))
                h = enc.tile([mi, B], F32R, tag=f"h2_{i}")
                nc.scalar.activation(h, ps, AF.Relu, bias=e2b_sb[i])
                h2_sb.append(h)

            ps_zm = psB.tile([L, B], F32, tag="psB")
            for j, (ko, ki) in enumerate(MT):
                nc.tensor.matmul(ps_zm, lhsT=zmw_sb[j], rhs=h2_sb[j],
                                 start=(j == 0), stop=(j == len(MT) - 1))
            zmean_sb = enc.tile([L, B], F32, tag="zmean")
            nc.vector.tensor_scalar_add(zmean_sb, ps_zm, zmb_sb)
            nc.sync.dma_start(out=z_meanT_out, in_=zmean_sb)

            ps_zv = psB.tile([L, B], F32, tag="psB")
            for j, (ko, ki) in enumerate(MT):
                nc.tensor.matmul(ps_zv, lhsT=zvw_sb[j], rhs=h2_sb[j],
                                 start=(j == 0), stop=(j == len(MT) - 1))
            zlv_sb = enc.tile([L, B], F32, tag="zlv")
            nc.vector.tensor_scalar_add(zlv_sb, ps_zv, zvb_sb)
            nc.sync.dma_start(out=z_log_varT_out, in_=zlv_sb)

            e_sb = enc.tile([L, B], F32, tag="e")
            nc.scalar.activation(e_sb, zlv_sb, AF.Exp, scale=0.5)
            prod_sb = enc.tile([L, B], F32, tag="prod")
            nc.vector.tensor_mul(prod_sb, e_sb, epsT_sb)
            z_sb = enc.tile([L, B], F32, tag="z")
            nc.vector.tensor_add(z_sb, prod_sb, zmean_sb)
            nc.sync.dma_start(out=zT_out, in_=z_sb)

            # ---- decode: 32 blocks of 2 d-columns x 256 batch --------
            # Per-block order g1 -> g2 -> head, but each block's maskedT is
            # produced at the END of the previous block so ScalarE services
            # the g1 relus first and the mask hides under g2/head matmuls.
            mk_store = {}

            def do_mask(blk):
                # maskedT = z * W-column, per-partition scale on ScalarE
                mk = maskp.tile([L, NB], DEC, tag="mask")
                for t in range(DPB):
                    d = blk * DPB + t
                    # VectorE 2x-mode tensor_scalar: ~2.5x cheaper than the
                    # ScalarE scaled-copy, and keeps ScalarE free for relus
                    nc.vector.tensor_scalar_mul(
                        mk[:, t * B : (t + 1) * B], z_sb, wl_sb[:, d : d + 1]
                    )
                mk_store[blk] = mk

            def do_g1(blk):
                mk = mk_store.pop(blk)
                g1_sb = []
                for i, (mo, mi) in enumerate(MT):
                    ps = psA.tile([mi, NB], F32, tag="psA")
                    nc.tensor.matmul(ps, lhsT=g1w_sb[:, mo : mo + mi], rhs=mk,
                                     start=True, stop=True)
                    t_ = g1p.tile([mi, NB], DEC, tag=f"g1_{i}")
                    nc.scalar.activation(t_, ps, AF.Relu)
                    g1_sb.append(t_)
                return g1_sb

            g2_store = {}

            def do_g2(blk, g1_sb):
                g2_sb = []
                for i, (mo, mi) in enumerate(MT):
                    ps = psB.tile([mi, NB], F32, tag="psB")
                    for j, (ko, ki) in enumerate(MT):
                        nc.tensor.matmul(
                            ps, lhsT=g2w_sb[j][:, mo : mo + mi], rhs=g1_sb[j],
                            start=(j == 0), stop=(j == len(MT) - 1))
                    t_ = g2p.tile([mi, NB], DEC, tag=f"g2_{i}")
                    # relu(psum + gen2_b) on the vector engine
                    nc.vector.tensor_scalar(
                        t_, ps, g2b_sb[i], 0.0, op0=ALU.add, op1=ALU.max
                    )
                    g2_sb.append(t_)
                g2_store[blk] = g2_sb

            def do_head(blk):
                g2_sb = g2_store.pop(blk)
                psh = psH.tile([DL, NB], F32, tag="psH")
                for j, (ko, ki) in enumerate(MT):
                    nc.tensor.matmul(psh, lhsT=hw_sb[j], rhs=g2_sb[j],
                                     start=(j == 0), stop=(j == len(MT) - 1))
                # PSUM is not DMA-readable: stage through SBUF via VectorE,
                # then DMA out the two diagonal rows of this block
                hs = hstp.tile([DL, NB], F32, tag="hstage")
                nc.scalar.activation(hs, psh, AF.Copy)
                for t in range(DPB):
                    d = blk * DPB + t
                    nc.sync.dma_start(
                        out=x_meanT[d : d + 1, :],
                        in_=hs[d : d + 1, t * B : (t + 1) * B],
                    )

            # PE order per iteration: [g1(blk) | head(blk-1) | g2(blk)].
            # head(blk-1)'s ~0.7us of matmuls sit between g1(blk) and
            # g2(blk), exactly covering the g1-relu -> g2-matmul latency;
            # all of head(blk-1)'s inputs are a block old. The next mask
            # is produced after this block's relus on the scalar engine.
            do_mask(0)
            for blk in range(NBLOCKS):
                g1_sb = do_g1(blk)
                if blk + 1 < NBLOCKS:
                    do_mask(blk + 1)
                if blk > 0:
                    do_head(blk - 1)
                do_g2(blk, g1_sb)
            do_head(NBLOCKS - 1)

    nc.compile()
    return nc


_NC_CACHE = None


def _get_nc():
    global _NC_CACHE
    if _NC_CACHE is None:
        _NC_CACHE = build_program()
    return _NC_CACHE


def make_in_maps(inputs):
    f = lambda a: np.ascontiguousarray(np.asarray(a), dtype=np.float32)
    shared = {
        "xT": f(inputs["x"].T),
        "epsT": f(inputs["eps"].T),
        "enc1_wT": f(inputs["enc1_w"].T),
        "enc1_b": f(inputs["enc1_b"].reshape(H, 1)),
        "enc2_wT": f(inputs["enc2_w"].T),
        "enc2_b": f(inputs["enc2_b"].reshape(H, 1)),
        "zm_wT": f(inputs["zm_w"].T),
        "zm_b": f(inputs["zm_b"].reshape(L, 1)),
        "zv_wT": f(inputs["zv_w"].T),
        "zv_b": f(inputs["zv_b"].reshape(L, 1)),
        "gen1_wT": f(inputs["gen1_w"].T),
        "gen2_wT": f(inputs["gen2_w"].T),
        "gen2_b": f(inputs["gen2_b"].reshape(H, 1)),
    }
    W = f(inputs["W"])
    head_w = f(inputs["head_w"])
    in_maps = []
    for c in range(NCORES):
        m = dict(shared)
        m["W_lT"] = f(W[c * DL : (c + 1) * DL, :].T)
        m["head_wT"] = f(head_w[c * DL : (c + 1) * DL, :].T)
        in_maps.append(m)
    return in_maps


def run_spmd(inputs, **kwargs):
    nc = _get_nc()
    return run_bass_kernel_spmd(nc, make_in_maps(inputs), list(range(NCORES)), **kwargs)


def assemble_outputs(results, inputs):
    x_mean = np.concatenate(
        [results[c]["x_meanT"].T for c in range(NCORES)], axis=1
    ) + np.asarray(inputs["head_b"], np.float32)[None, :]
    z_mean = results[0]["z_meanT"].T.copy()
    z_log_var = results[0]["z_log_varT"].T.copy()
    # recompute the reparameterization on host: exact fp32 exp (the on-chip
    # ACT Exp is an approximation; decode used the on-chip z)
    z = z_mean + np.asarray(inputs["eps"], np.float32) * np.exp(
        0.5 * z_log_var
    ).astype(np.float32)
    return x_mean, z, z_mean, z_log_var


def kernel(**inputs):
    res = run_spmd(inputs)
    return assemble_outputs(res.results, inputs)


if __name__ == "__main__":
    rng = np.random.default_rng(0)
    fake = {
        "x": rng.standard_normal((B, D), dtype=np.float32),
        "eps": rng.standard_normal((B, L), dtype=np.float32),
        "W": rng.standard_normal((D, L), dtype=np.float32) * 0.05,
        "enc1_w": rng.standard_normal((H, D), dtype=np.float32) * 0.05,
        "enc1_b": rng.standard_normal((H,), dtype=np.float32) * 0.05,
        "enc2_w": rng.standard_normal((H, H), dtype=np.float32) * 0.05,
        "enc2_b": rng.standard_normal((H,), dtype=np.float32) * 0.05,
        "zm_w": rng.standard_normal((L, H), dtype=np.float32) * 0.05,
        "zm_b": rng.standard_normal((L,), dtype=np.float32) * 0.05,
        "zv_w": rng.standard_normal((L, H), dtype=np.float32) * 0.05,
        "zv_b": rng.standard_normal((L,), dtype=np.float32) * 0.05,
        "gen1_w": rng.standard_normal((H, L), dtype=np.float32) * 0.05,
        "gen2_w": rng.standard_normal((H, H), dtype=np.float32) * 0.05,
        "gen2_b": rng.standard_normal((H,), dtype=np.float32) * 0.05,
        "head_w": rng.standard_normal((D, H), dtype=np.float32) * 0.05,
        "head_b": rng.standard_normal((D,), dtype=np.float32) * 0.05,
    }
    outs = kernel(**fake)
    for name, o in zip(("x_mean", "z", "z_mean", "z_log_var"), outs):
        print(name, o.shape, o.dtype, float(np.abs(o).max()))
